# revision 9
# baseline (speedup 1.0000x reference)
"""nn_Head single-head causal attention on 8 TRN2 NeuronCores.

Full inputs: x [8, 2048, 1024] f32, Wk/Wq/Wv [1024, 64] f32.
Full output: [8, 2048, 64] f32 = softmax(causal(q k^T * C^-0.5)) @ v per batch.

Sharding: data-parallel over batch B=8 -> one batch element per core;
weights replicated. No collectives.

Per-core kernel (Bass/Tile, f32r matmuls + bf16 probability/value stage):
  A) load x t-tiles, PE-transpose to xT [c-part, t] (TensorE contracts over
     the partition dim, so x must be c-major; fp32 cannot DMA-transpose)
  B) QKV: kT/qT/vT [h(64), t] via lhsT=W [c,64], rhs=xT; v transposed back
     to natural v1 [s-part, t-tile, H+1] bf16 with a ones column at H that
     makes the PV matmul also produce the softmax denominator
  C) S^T tile = kT_slice^T@qT -> PSUM [s 128, t 512]; exp on ScalarE with
     scale=C^-0.5 folded in (scores are O(1): no max-subtraction needed,
     mathematically identical softmax); causality via memset of fully-masked
     column ranges + a 0/1 upper-triangular mask mul on diagonal tiles;
     PV: po[h|denom, t] += v1_slice^T @ P^T accumulated over s-tiles
  E) po -> SBUF, PE-transpose to [t-part, H+1], multiply by per-partition
     reciprocal of the denominator column, single output DMA.
"""

from contextlib import ExitStack

import numpy as np

import concourse.bass as bass
import concourse.mybir as mybir
import concourse.tile as tile
from concourse import bass_utils
from concourse.masks import make_identity

B, T, C, H = 8, 2048, 1024, 64
N_CORES = 8
P = 128


def _patch_drain_split():
    """This walrus build accepts only one sem wait per instruction ("Too many
    sync wait commands" in setupSyncWait otherwise). Hoist extra waits onto
    same-engine NOPs ahead of the instruction (engine streams dispatch
    in-order, so the blocking semantics are identical), and split the
    TileContext tail drain the same way."""
    if getattr(tile.TileContext, "_drain_split_patched", False):
        return
    from concourse.tile import ScopedClock

    _orig_add = tile.TileContext._add_instruction

    def _patched_add(self, inst):
        si = getattr(inst, "sync_info", None)
        if si is not None and si.on_wait and len(si.on_wait) > 1:
            waits = list(si.on_wait)
            for i, w in enumerate(waits[:-1]):
                nop = mybir.InstNoOp(
                    name=f"{inst.name}-ws{i}",
                    sync_info=mybir.SyncInfo(on_wait=[w], on_update=[]),
                    bass_nofuse=True,
                    engine=inst.engine,
                )
                _orig_add(self, nop)
            si.on_wait = waits[-1:]
            inst.sync_info = si
        _orig_add(self, inst)

    tile.TileContext._add_instruction = _patched_add

    def _patched_dab(self, tick_clock, wait_clock):
        nc = self.nc
        drain_inst = nc.sync.drain()
        wait_clock.add_sem_waits(
            drain_inst.ins, ScopedClock({None: tick_clock.global_clock})
        )
        si = drain_inst.ins.sync_info
        if si is not None and si.on_wait and len(si.on_wait) > 1:
            waits = list(si.on_wait)
            si.on_wait = waits[:1]
            drain_inst.ins.sync_info = si
            for w in waits[1:]:
                d2 = nc.sync.drain()
                d2.ins.sync_info = mybir.SyncInfo(on_wait=[w], on_update=[])
        nc.all_engine_barrier()
        popped = nc._tile_sem_poison_stack.pop()
        assert popped is self._sem_poison
        nc.clear_and_free_semaphores(list(self.sems.allocated().values()))
        nc.all_engine_barrier()

    tile.TileContext._drain_and_barrier = _patched_dab
    tile.TileContext._drain_split_patched = True


def _emit(tc, out_d, x_d, wk_d, wq_d, wv_d):
    nc = tc.nc
    f32r = mybir.dt.float32r
    f32 = mybir.dt.float32
    bf16 = mybir.dt.bfloat16
    Exp = mybir.ActivationFunctionType.Exp

    CT = C // P  # 8 c-tiles
    TT = T // P  # 16 t-tiles
    BLK = 512
    NB = T // BLK  # 4 t-blocks
    SPB = BLK // P  # 4 s-tiles per block width
    H1 = H + 1
    scale = float(C) ** -0.5

    with ExitStack() as ctx:
        const = ctx.enter_context(tc.tile_pool(name="const", bufs=1))
        persist = ctx.enter_context(tc.tile_pool(name="persist", bufs=1))
        xa_pool = ctx.enter_context(tc.tile_pool(name="xa", bufs=6))
        pt_pool = ctx.enter_context(tc.tile_pool(name="ptp", bufs=4))
        oT_pool = ctx.enter_context(tc.tile_pool(name="otp", bufs=2))
        rec_pool = ctx.enter_context(tc.tile_pool(name="recp", bufs=2))
        # PSUM: 8 banks total so all phases can overlap.
        psA = ctx.enter_context(tc.tile_pool(name="psA", bufs=1, space="PSUM"))
        psB = ctx.enter_context(tc.tile_pool(name="psB", bufs=2, space="PSUM"))
        psS = ctx.enter_context(tc.tile_pool(name="psS", bufs=2, space="PSUM"))
        psOE = ctx.enter_context(tc.tile_pool(name="psOE", bufs=1, space="PSUM"))

        # identity: build in f32 (memset on f32r is invalid ISA in this
        # walrus), keep an f32r copy for same-dtype transposes
        ident = const.tile([P, P], f32, name="ident")
        make_identity(nc, ident)
        identr = const.tile([P, P], f32r, name="identr")
        nc.vector.tensor_copy(out=identr, in_=ident)
        # 0/1 mask: mask[s, t] = 1 iff s <= t (keep causal entries)
        mask = const.tile([P, P], bf16, name="mask")
        nc.vector.memset(mask, 1.0)
        nc.gpsimd.affine_select(
            out=mask,
            in_=mask,
            compare_op=mybir.AluOpType.is_ge,
            fill=0.0,
            base=0,
            pattern=[[1, P]],
            channel_multiplier=-1,
        )

        # [Wk | Wq] packed: one M=128 matmul produces k on partitions 0-63
        # and q on 64-127
        wkq_sb = const.tile([P, CT, 2 * H], f32r, name="wkq_sb")
        wv_sb = const.tile([P, CT, H], f32r, name="wv_sb")

        xT = persist.tile([P, CT, T], f32r, name="xT")
        kT = persist.tile([H, T], f32r, name="kT")
        qT = persist.tile([H, T], f32r, name="qT")
        vT = persist.tile([H, T], f32, name="vT")
        v1 = persist.tile([P, TT, H1], bf16, name="v1")
        out_sb = persist.tile([P, TT, H], f32, name="out_sb")

        nc.vector.memset(v1[:, :, H : H + 1], 1.0)

        # Phase A: x -> xT via PE transpose. Weight DMAs are emitted after
        # the first x tiles so they don't delay the transpose pipeline.
        for tt in range(TT):
            tsl = slice(tt * P, (tt + 1) * P)
            xa = xa_pool.tile([P, C], f32r, name="xa")
            nc.sync.dma_start(xa, x_d[tsl, :])
            if tt == 3:
                nc.sync.dma_start(
                    wkq_sb[:, :, 0:H], wk_d.rearrange("(o p) h -> p o h", p=P)
                )
                nc.sync.dma_start(
                    wkq_sb[:, :, H : 2 * H],
                    wq_d.rearrange("(o p) h -> p o h", p=P),
                )
                nc.sync.dma_start(
                    wv_sb, wv_d.rearrange("(o p) h -> p o h", p=P)
                )
            for cg in range(CT // 4):
                ps_t = psA.tile([P, 4, P], f32r, name="ps_t")
                for j in range(4):
                    ci = cg * 4 + j
                    nc.tensor.transpose(
                        ps_t[:, j, :], xa[:, ci * P : (ci + 1) * P], identr
                    )
                dst = xT[:, cg * 4 : cg * 4 + 4, tsl]
                if (tt + cg) % 2 == 0:
                    nc.vector.tensor_copy(out=dst, in_=ps_t)
                else:
                    nc.scalar.copy(out=dst, in_=ps_t)

        # Phase B: QKV projections (kq packed) + v back to natural layout
        for bi in range(NB):
            tsl = slice(bi * BLK, (bi + 1) * BLK)
            pkq = psB.tile([P, BLK], f32, name="pkq", tag="qkv")
            for ci in range(CT):
                nc.tensor.matmul(
                    pkq,
                    wkq_sb[:, ci, :],
                    xT[:, ci, tsl],
                    start=(ci == 0),
                    stop=(ci == CT - 1),
                )
            nc.vector.tensor_copy(out=kT[:, tsl], in_=pkq[0:H, :])
            # partition-shift copy 64-127 -> 0-63 (legal on DVE)
            nc.vector.tensor_copy(out=qT[:, tsl], in_=pkq[H:P, :])
            pv = psB.tile([H, BLK], f32, name="pv", tag="qkv")
            for ci in range(CT):
                nc.tensor.matmul(
                    pv,
                    wv_sb[:, ci, :],
                    xT[:, ci, tsl],
                    start=(ci == 0),
                    stop=(ci == CT - 1),
                )
            nc.vector.tensor_copy(out=vT[:, tsl], in_=pv)
            for c4 in range(SPB):
                st = bi * SPB + c4
                pvt = psB.tile([P, H], f32, name="pvt", tag="qkv")
                nc.tensor.transpose(
                    pvt, vT[:, st * P : (st + 1) * P], ident[:H, :H]
                )
                nc.vector.tensor_copy(out=v1[:, st, 0:H], in_=pvt)

        # Phase C: attention
        for bi in range(NB):
            tsl = slice(bi * BLK, (bi + 1) * BLK)
            po = psOE.tile([H1, BLK], f32, name="po", tag="poe")
            NS = SPB * (bi + 1)
            for g in range(NS // 2):
                ps_s = psS.tile([P, 2, BLK], f32, name="ps_s")
                for j in range(2):
                    st = 2 * g + j
                    nc.tensor.matmul(
                        ps_s[:, j, :],
                        kT[:, st * P : (st + 1) * P],
                        qT[:, tsl],
                        start=True,
                        stop=True,
                    )
                ptile = pt_pool.tile([P, 2, BLK], bf16, name="ptile")
                d0s = [max(0, (2 * g + j) * P - bi * BLK) for j in range(2)]
                if d0s[0] == 0 and d0s[1] == 0:
                    nc.scalar.activation(ptile, ps_s, Exp, scale=scale)
                else:
                    # skip fully-masked prefix columns: exp only the valid
                    # suffix, zero the prefix on DVE
                    for j in range(2):
                        d0 = d0s[j]
                        nc.scalar.activation(
                            ptile[:, j, d0:], ps_s[:, j, d0:], Exp, scale=scale
                        )
                        if d0 > 0:
                            nc.vector.memset(ptile[:, j, 0:d0], 0.0)
                for j in range(2):
                    st = 2 * g + j
                    d0 = st * P - bi * BLK
                    if d0 >= 0:  # tile touches/precedes the diagonal
                        nc.vector.tensor_mul(
                            ptile[:, j, d0 : d0 + P],
                            ptile[:, j, d0 : d0 + P],
                            mask,
                        )
                for j in range(2):
                    st = 2 * g + j
                    nc.tensor.matmul(
                        po,
                        v1[:, st, 0:H1],
                        ptile[:, j, :],
                        start=(st == 0),
                        stop=(st == NS - 1),
                    )

            oT = oT_pool.tile([H1, BLK], f32, name="oT")
            nc.vector.tensor_copy(out=oT, in_=po)
            for c4 in range(SPB):
                pe = psOE.tile([P, H1], f32, name="pe", tag="poe")
                nc.tensor.transpose(
                    pe, oT[:, c4 * P : (c4 + 1) * P], ident[:H1, :H1]
                )
                rec = rec_pool.tile([P, 1], f32, name="rec")
                nc.vector.reciprocal(rec, pe[:, H:H1])
                nc.vector.tensor_scalar_mul(
                    out_sb[:, bi * SPB + c4, :], pe[:, 0:H], rec
                )
            # stream this block's rows out while later blocks compute
            nc.sync.dma_start(
                out_d.rearrange("(o p) h -> p o h", p=P)[
                    :, bi * SPB : (bi + 1) * SPB, :
                ],
                out_sb[:, bi * SPB : (bi + 1) * SPB, :],
            )


_NC_CACHE = {}


def build_nc():
    if "nc" in _NC_CACHE:
        return _NC_CACHE["nc"]
    _patch_drain_split()
    f32r = mybir.dt.float32r
    f32 = mybir.dt.float32
    nc = bass.Bass(
        "TRN2", target_bir_lowering=False, debug=False, num_devices=N_CORES
    )
    x_d = nc.dram_tensor("x", [T, C], f32r, kind="ExternalInput").ap()
    wk_d = nc.dram_tensor("Wk", [C, H], f32r, kind="ExternalInput").ap()
    wq_d = nc.dram_tensor("Wq", [C, H], f32r, kind="ExternalInput").ap()
    wv_d = nc.dram_tensor("Wv", [C, H], f32r, kind="ExternalInput").ap()
    out_d = nc.dram_tensor("out", [T, H], f32, kind="ExternalOutput").ap()
    with tile.TileContext(nc) as tc:
        _emit(tc, out_d, x_d, wk_d, wq_d, wv_d)
    _NC_CACHE["nc"] = nc
    return nc


def kernel(x, Wk, Wq, Wv, **run_kwargs):
    """Full-input entry point: shard over batch, run on cores 0-7, gather."""
    x = np.ascontiguousarray(np.asarray(x), dtype=np.float32)
    Wk = np.ascontiguousarray(np.asarray(Wk), dtype=np.float32)
    Wq = np.ascontiguousarray(np.asarray(Wq), dtype=np.float32)
    Wv = np.ascontiguousarray(np.asarray(Wv), dtype=np.float32)
    assert x.shape == (B, T, C), x.shape

    nc = build_nc()
    in_maps = [
        {"x": np.ascontiguousarray(x[b]), "Wk": Wk, "Wq": Wq, "Wv": Wv}
        for b in range(B)
    ]
    res = bass_utils.run_bass_kernel_spmd(
        nc, in_maps, core_ids=list(range(N_CORES)), **run_kwargs
    )
    out = np.stack([res.results[b]["out"] for b in range(B)], axis=0)
    if run_kwargs:
        kernel.last_results = res
    return out.astype(np.float32)
